# revision 1
# baseline (speedup 1.0000x reference)
"""Multi-head attention (B=8, T=2048, D=512, H=8) on 8 TRN2 NeuronCores.

Sharding: data-parallel over batch — one batch element per core, no
collectives. Host-side prep (part of shard/unshard): transpose x inputs to
[D, T], cast matmul operands to bf16, pass (1 - mask)^T chunk-major, and
transpose the per-core output y^T back to [T, D].

Per-core algorithm ("transposed flash", everything in one PE tiling mode):
  P1: Q^T = Wq x^T, K^T = Wk x^T (padded per-head into zero-padded 128-row
      tiles), V = x Wv^T (augmented with a ones column per head for the
      softmax denominator).
  P2: per (q-block, head, t2-chunk):
        S^T[t2,q] = Kpad_h^T.T @ Q^T          (PSUM, scale deferred)
        P_raw     = exp(S^T / 8)              (ScalarE, PSUM -> SBUF bf16)
        P         = P_raw * (1-mask)^T        (VectorE; equals reference's
                                               where(mask,-inf) + where(mask,0)
                                               since exp(-1e4) == 0 in f32)
        O_aug^T  += Vaug_h.T @ P              (PSUM accum; row 64 = denom)
      epilogue: recip(denom) -> broadcast -> O^T = O_aug^T[0:64] * recip.
  P3: y^T = Wo^T.T @ O^T (+bo), DMA out.

Biases bq, bk, bo are applied (per-partition fused adds); bv via a
broadcast add on V eviction. No max-subtraction in softmax: scores are
O(6) so exp is safe in f32, matching the reference to ~bf16 accuracy.
"""

import numpy as np
import ml_dtypes

B, T, FDIM, H = 8, 2048, 512, 8
DK = FDIM // H          # 64
NFT = FDIM // 128       # 4 fo-tiles
NCH = T // 128          # 16 t2-chunks
QB = 2                  # q blocks
QBS = T // QB           # 1024
N_CORES = 8

BF16 = ml_dtypes.bfloat16

_cache = {}


def _build_nc():
    import concourse.bass as bass
    import concourse.mybir as mybir
    from concourse import bacc, tile

    f32 = mybir.dt.float32
    bf16 = mybir.dt.bfloat16
    Exp = mybir.ActivationFunctionType.Exp
    Alu = mybir.AluOpType

    nc = bacc.Bacc("TRN2", target_bir_lowering=False, debug=False,
                   num_devices=N_CORES)

    # DRAM I/O (per-core shard shapes)
    xqT = nc.dram_tensor("xqT", [FDIM, T], bf16, kind="ExternalInput")
    xkT = nc.dram_tensor("xkT", [FDIM, T], bf16, kind="ExternalInput")
    xvT = nc.dram_tensor("xvT", [FDIM, T], bf16, kind="ExternalInput")
    wqT = nc.dram_tensor("wqT", [FDIM, FDIM], bf16, kind="ExternalInput")
    wkT = nc.dram_tensor("wkT", [FDIM, FDIM], bf16, kind="ExternalInput")
    wvT = nc.dram_tensor("wvT", [FDIM, FDIM], bf16, kind="ExternalInput")
    woT = nc.dram_tensor("woT", [FDIM, FDIM], bf16, kind="ExternalInput")
    bq = nc.dram_tensor("bq", [FDIM], f32, kind="ExternalInput")
    bk = nc.dram_tensor("bk", [FDIM], f32, kind="ExternalInput")
    bv = nc.dram_tensor("bv", [FDIM], f32, kind="ExternalInput")
    bo = nc.dram_tensor("bo", [FDIM], f32, kind="ExternalInput")
    mbar = nc.dram_tensor("mbar", [NCH, 128, T], bf16, kind="ExternalInput")
    yT = nc.dram_tensor("yT", [FDIM, T], f32, kind="ExternalOutput")
    # DRAM bounce rows for partition-broadcasting softmax reciprocals
    rscratch = nc.dram_tensor("rscratch", [QB * H, QBS], f32)

    import os
    dbg = os.environ.get("KERNEL_DEBUG_DUMPS") == "1"
    if dbg:
        dbg_qt = nc.dram_tensor("dbg_qt", [128, T], bf16, kind="ExternalOutput")
        dbg_kp = nc.dram_tensor("dbg_kp", [2, 128, T], bf16, kind="ExternalOutput")
        dbg_va = nc.dram_tensor("dbg_va", [128, H * (DK + 1)], bf16, kind="ExternalOutput")
        dbg_pm = nc.dram_tensor("dbg_pm", [128, QBS], bf16, kind="ExternalOutput")
        dbg_osb = nc.dram_tensor("dbg_osb", [64, QBS], bf16, kind="ExternalOutput")
        dbg_rb = nc.dram_tensor("dbg_rb", [2, QBS], f32, kind="ExternalOutput")

    _dma_rr = [0]

    with tile.TileContext(nc) as tc:
        def bulk_dma(out, in_):
            nc.sync.dma_start(out=out, in_=in_)

        with (
            tc.tile_pool(name="consts", bufs=1) as consts,
            tc.tile_pool(name="qt", bufs=1) as qt_pool,
            tc.tile_pool(name="kpad", bufs=1) as kpad_pool,
            tc.tile_pool(name="vaug", bufs=1) as vaug_pool,
            tc.tile_pool(name="osb", bufs=1) as osb_pool,
            tc.tile_pool(name="ysb", bufs=1) as ysb_pool,
        ):
            # ---- consts: weights + biases ----
            wq_sb = [consts.tile([128, FDIM], bf16, tag=f"wq{fc}", name=f"wq{fc}") for fc in range(4)]
            wk_sb = [consts.tile([128, FDIM], bf16, tag=f"wk{fc}", name=f"wk{fc}") for fc in range(4)]
            wv_sb = [consts.tile([128, FDIM], bf16, tag=f"wv{fc}", name=f"wv{fc}") for fc in range(4)]
            wo_sb = [consts.tile([128, FDIM], bf16, tag=f"wo{j}", name=f"wo{j}") for j in range(NFT)]
            # wv first: the V projection is the head of the critical path
            for fc in range(4):
                bulk_dma(out=wv_sb[fc][:], in_=wvT[fc * 128:(fc + 1) * 128, :])
            for fc in range(4):
                bulk_dma(out=wq_sb[fc][:], in_=wqT[fc * 128:(fc + 1) * 128, :])
                bulk_dma(out=wk_sb[fc][:], in_=wkT[fc * 128:(fc + 1) * 128, :])
            for j in range(NFT):
                bulk_dma(out=wo_sb[j][:], in_=woT[j * 128:(j + 1) * 128, :])

            bq_sb = consts.tile([128, NFT], f32, tag="bq", name="bq")
            bk_sb = consts.tile([128, NFT], f32, tag="bk", name="bk")
            bo_sb = consts.tile([128, NFT], f32, tag="bo", name="bo")
            for b_dram, b_t in ((bq, bq_sb), (bk, bk_sb), (bo, bo_sb)):
                nc.sync.dma_start(out=b_t[:], in_=b_dram.ap().rearrange("(j p) -> p j", p=128))
            bv_bcast = consts.tile([128, FDIM], f32, tag="bv_bcast", name="bv_bcast")
            nc.sync.dma_start(
                out=bv_bcast[:],
                in_=bv.ap().rearrange("(a f) -> a f", a=1).to_broadcast([128, FDIM]))

            # ---- persistent activation tiles ----
            qT_sb = [qt_pool.tile([128, T], bf16, tag=f"qT{j}", name=f"qT{j}") for j in range(NFT)]
            kpad = [kpad_pool.tile([128, T], bf16, tag=f"kp{h}", name=f"kp{h}") for h in range(H)]
            # zero the unused head-half of each padded K tile, once
            for h in range(H):
                half = slice(64, 128) if h % 2 == 0 else slice(0, 64)
                nc.vector.memset(kpad[h][half, :], 0.0)
            vaug = [vaug_pool.tile([128, H * (DK + 1)], bf16, tag=f"va{tt}", name=f"va{tt}")
                    for tt in range(NCH)]
            # ones column per head in V_aug
            for tt in range(NCH):
                va = vaug[tt][:].rearrange("p (h d) -> p h d", d=DK + 1)
                nc.vector.memset(va[:, :, DK:DK + 1], 1.0)

            o2_sb = {}
            for qb in range(QB):
                for j in range(NFT):
                    o2_sb[(qb, j)] = osb_pool.tile([128, QBS], bf16, tag=f"o2_{qb}_{j}",
                                                   name=f"o2_{qb}_{j}")

            # ============ PSUM pool (8 banks total, shared by phases) ======
            # tag "s":  2 x [128,1024] f32 = 4 banks  (scores / QK-proj)
            # tag o0/o1: 1 x [128,1024]-sized slot each = 4 banks
            #            (attnV accumulators, V-proj, P3 y-tiles)
            with (
                tc.tile_pool(name="xt", bufs=5) as xt_pool,
                tc.tile_pool(name="mask", bufs=16) as mask_pool,
                tc.tile_pool(name="praw", bufs=3) as praw_pool,
                tc.tile_pool(name="pm", bufs=3) as pm_pool,
                tc.tile_pool(name="rb", bufs=1) as rb_pool,
                tc.tile_pool(name="psum", bufs=2, space="PSUM") as psum_pool,
            ):
                def load_xT(xT_dram, tag):
                    tiles = []
                    for fc in range(4):
                        xt = xt_pool.tile([128, T], bf16, tag=tag, bufs=4, name="xt")
                        bulk_dma(out=xt[:], in_=xT_dram[fc * 128:(fc + 1) * 128, :])
                        tiles.append(xt)
                    return tiles

                def v_proj_tile(tt, ptag):
                    if True:
                        ps = psum_pool.tile([128, 512], mybir.dt.float32,
                                            tag=ptag, bufs=1, name="vp")
                        for fc in range(4):
                            nc.tensor.matmul(
                                ps[:],
                                xts_v[fc][:, tt * 128:(tt + 1) * 128],
                                wv_sb[fc][:],
                                start=(fc == 0), stop=(fc == 3),
                            )
                        va = vaug[tt][:].rearrange("p (h d) -> p h d", d=DK + 1)
                        nc.vector.scalar_tensor_tensor(
                            out=va[:, :, 0:DK],
                            in0=ps[:].rearrange("p (h d) -> p h d", d=DK),
                            scalar=1.0,
                            in1=bv_bcast[:].rearrange("p (h d) -> p h d", d=DK),
                            op0=Alu.mult, op1=Alu.add,
                        )

                def proj_groups(j, xts, w_sb, b_t, dst, slices):
                    for s in slices:
                        ps = psum_pool.tile([128, 512], mybir.dt.float32,
                                            tag="s", name="qkp")
                        for fc in range(4):
                            nc.tensor.matmul(
                                ps[:],
                                w_sb[fc][:, j * 128:(j + 1) * 128],
                                xts[fc][:, s * 512:(s + 1) * 512],
                                start=(fc == 0), stop=(fc == 3),
                            )
                        sl = slice(s * 512, (s + 1) * 512)
                        if dst is not None:
                            nc.vector.tensor_scalar_add(dst[:, sl], ps[:],
                                                        b_t[:, j:j + 1])
                        else:  # K: evict into the two padded per-head tiles
                            nc.vector.tensor_scalar_add(
                                kpad[2 * j][0:64, sl], ps[0:64, :],
                                b_t[0:64, j:j + 1])
                            nc.vector.tensor_scalar_add(
                                kpad[2 * j + 1][64:128, sl], ps[64:128, :],
                                b_t[64:128, j:j + 1])

                def q_proj(j, half):
                    proj_groups(j, xts_q, wq_sb, bq_sb, qT_sb[j],
                                range(2 * half, 2 * half + 2))

                def k_proj(j, half):
                    proj_groups(j, xts_k, wk_sb, bk_sb, None,
                                range(2 * half, 2 * half + 2))

                def p3(qb):
                    qsl = slice(qb * QBS, (qb + 1) * QBS)
                    for i in range(NFT):
                        y_ps = psum_pool.tile([128, QBS], mybir.dt.float32,
                                              tag=f"o{i % 2}", bufs=1, name="y")
                        for j in range(NFT):
                            for s in range(2):
                                nc.tensor.matmul(
                                    y_ps[:, s * 512:(s + 1) * 512],
                                    wo_sb[j][:, i * 128:(i + 1) * 128],
                                    o2_sb[(qb, j)][:, s * 512:(s + 1) * 512],
                                    start=(j == 0), stop=(j == NFT - 1),
                                )
                        y_sb = ysb_pool.tile([128, QBS], mybir.dt.float32, tag="ysb", name="ysb")
                        nc.vector.tensor_scalar_add(y_sb[:], y_ps[:], bo_sb[:, i:i + 1])
                        nc.sync.dma_start(out=yT[i * 128:(i + 1) * 128, qsl], in_=y_sb[:])

                # ---- P1 prefix: V first (every head needs it), then QK j=0
                xts_v = load_xT(xvT, "xq")  # slots reused by xq after v_proj
                for tt in range(NCH):
                    v_proj_tile(tt, f"o{tt % 2}")
                xts_k = load_xT(xkT, "xk")
                xts_q = load_xT(xqT, "xq")
                k_proj(0, 0)
                k_proj(0, 1)
                q_proj(0, 0)

                if dbg:
                    nc.sync.dma_start(out=dbg_qt.ap(), in_=qT_sb[0][:])
                    nc.sync.dma_start(out=dbg_kp.ap()[0], in_=kpad[0][:])
                    nc.sync.dma_start(out=dbg_kp.ap()[1], in_=kpad[1][:])
                    nc.sync.dma_start(out=dbg_va.ap(), in_=vaug[0][:])

                # ---- P2 + P3, with remaining QK projections interleaved ----
                for qb in range(QB):
                    qsl = slice(qb * QBS, (qb + 1) * QBS)
                    mask_t = []
                    for c in range(NCH):
                        mt = mask_pool.tile([128, QBS], bf16, tag="mask", name="mask")
                        nc.sync.dma_start(out=mt[:], in_=mbar[c, :, qsl])
                        mask_t.append(mt)

                    for h in range(H):
                        j = h // 2
                        o_ps = psum_pool.tile([DK + 1, QBS], mybir.dt.float32,
                                              tag=f"o{h % 2}", bufs=1, name="o")
                        for c in range(NCH):
                            s_ps = psum_pool.tile([128, QBS], mybir.dt.float32,
                                                  tag="s", name="s")
                            for s in range(2):
                                nc.tensor.matmul(
                                    s_ps[:, s * 512:(s + 1) * 512],
                                    kpad[h][:, c * 128:(c + 1) * 128],
                                    qT_sb[j][:, qb * QBS + s * 512: qb * QBS + (s + 1) * 512],
                                    start=True, stop=True,
                                )
                            p_raw = praw_pool.tile([128, QBS], bf16, tag="praw", name="praw")
                            nc.scalar.activation(p_raw[:], s_ps[:], Exp,
                                                 bias=0.0, scale=0.125)
                            p_m = pm_pool.tile([128, QBS], bf16, tag="pm", name="pm")
                            nc.vector.tensor_mul(p_m[:], p_raw[:], mask_t[c][:])
                            if dbg and qb == 0 and h == 0 and c == 0:
                                nc.sync.dma_start(out=dbg_pm.ap(), in_=p_m[:])
                            for s in range(2):
                                nc.tensor.matmul(
                                    o_ps[:, s * 512:(s + 1) * 512],
                                    vaug[c][:, h * (DK + 1):(h + 1) * (DK + 1)],
                                    p_m[:, s * 512:(s + 1) * 512],
                                    start=(c == 0), stop=(c == NCH - 1),
                                )
                        # epilogue: divide by the denominator (row DK of o_ps).
                        # reciprocal is ~8 cyc/elem/lane -> split the [1,1024]
                        # row over 8 partitions via SBUF->SBUF DMA; a DRAM
                        # bounce row broadcasts it across partitions 0-63.
                        rb = rb_pool.tile([128, QBS], mybir.dt.float32, tag="rb", name="rb")
                        rbs = rb_pool.tile([8, QBS // 8], mybir.dt.float32, tag="rbs", name="rbs")
                        rbr = rb_pool.tile([8, QBS // 8], mybir.dt.float32, tag="rbr", name="rbr")
                        nc.vector.tensor_copy(rb[64:65, :], o_ps[DK:DK + 1, :])
                        nc.sync.dma_start(out=rbs[:], in_=rb[64:65, :])
                        nc.vector.reciprocal(rbr[:], rbs[:])
                        rrow = rscratch.ap()[qb * H + h: qb * H + h + 1, :]
                        nc.sync.dma_start(out=rrow, in_=rbr[:])
                        nc.sync.dma_start(out=rb[0:64, :],
                                          in_=rrow.to_broadcast([64, QBS]))
                        osm = rb_pool.tile([64, QBS], bf16, tag="osm", bufs=3, name="osm")
                        nc.vector.tensor_mul(osm[:], o_ps[0:DK, :], rb[0:64, :])
                        nc.sync.dma_start(
                            out=o2_sb[(qb, h // 2)][(h % 2) * 64:(h % 2) * 64 + 64, :],
                            in_=osm[:])
                        if dbg and qb == 0 and h == 0:
                            nc.sync.dma_start(out=dbg_rb.ap()[0:1, :], in_=rb[0:1, :])
                            nc.sync.dma_start(out=dbg_rb.ap()[1:2, :], in_=rb[64:65, :])
                            nc.sync.dma_start(out=dbg_osb.ap(), in_=osm[:])

                        # overlap remaining projections with the attention
                        # stream: K(j) fully before head 2j; Q(j) per q-block.
                        steps = ()
                        if qb == 0:
                            steps = [(("k", 1, 0),),
                                     (("k", 1, 1), ("q", 1, 0)),
                                     (("k", 2, 0),), (("k", 2, 1), ("q", 2, 0)),
                                     (("k", 3, 0),), (("k", 3, 1), ("q", 3, 0)),
                                     (("q", 0, 1),), ()][h]
                        else:
                            steps = [(("q", 1, 1),), (("q", 2, 1),),
                                     (("q", 3, 1),)][h] if h < 3 else ()
                        for kind, jj, hh in steps:
                            if kind == "k":
                                k_proj(jj, hh)
                            else:
                                q_proj(jj, hh)
                        if qb == 1 and h == 0:
                            p3(0)


                p3(1)

    nc.compile()
    return nc


def _get_nc():
    if "nc" not in _cache:
        _cache["nc"] = _build_nc()
    return _cache["nc"]


def _make_in_maps(inputs):
    query = np.asarray(inputs["query"], np.float32)
    key = np.asarray(inputs["key"], np.float32)
    value = np.asarray(inputs["value"], np.float32)
    mask = np.asarray(inputs["mask"], bool)
    shared = {
        "wqT": np.ascontiguousarray(np.asarray(inputs["Wq"], np.float32).T).astype(BF16),
        "wkT": np.ascontiguousarray(np.asarray(inputs["Wk"], np.float32).T).astype(BF16),
        "wvT": np.ascontiguousarray(np.asarray(inputs["Wv"], np.float32).T).astype(BF16),
        "woT": np.ascontiguousarray(np.asarray(inputs["Wo"], np.float32).T).astype(BF16),
        "bq": np.asarray(inputs["bq"], np.float32),
        "bk": np.asarray(inputs["bk"], np.float32),
        "bv": np.asarray(inputs["bv"], np.float32),
        "bo": np.asarray(inputs["bo"], np.float32),
    }
    in_maps = []
    for b in range(N_CORES):
        m = dict(shared)
        m["xqT"] = np.ascontiguousarray(query[b].T).astype(BF16)
        m["xkT"] = np.ascontiguousarray(key[b].T).astype(BF16)
        m["xvT"] = np.ascontiguousarray(value[b].T).astype(BF16)
        mb = (~mask[b]).T.astype(BF16)          # (1 - mask)^T, [t2, q]
        m["mbar"] = np.ascontiguousarray(mb.reshape(NCH, 128, T))
        in_maps.append(m)
    return in_maps


def run(inputs, trace=False, **kwargs):
    from concourse.bass_utils import run_bass_kernel_spmd
    nc = _get_nc()
    res = run_bass_kernel_spmd(nc, _make_in_maps(inputs),
                               core_ids=list(range(N_CORES)),
                               trace=trace, **kwargs)
    y = np.stack([np.asarray(res.results[b]["yT"], np.float32).T
                  for b in range(N_CORES)])
    return y, res


def kernel(**inputs) -> np.ndarray:
    y, _ = run(inputs, trace=False)
    return y



# revision 8
# speedup vs baseline: 1.0663x; 1.0663x over previous
"""Multi-head attention (B=8, T=2048, D=512, H=8) on 8 TRN2 NeuronCores.

Sharding: data-parallel over batch - one batch element per core, no
collectives. Host-side prep: transpose x inputs to [D, T], cast matmul
operands to bf16, pass (1 - mask)^T chunk-major, transpose per-core y^T
back to [T, D].

The kernel is organized around keeping the Scalar (activation) engine's
exp stream gapless: 256 exp slices of [128, 1024] at ~1.2us each are the
hard floor (~307us). Everything else (projections, attn-V, output proj,
epilogues) is scheduled into the PE/DVE/GpSimd/DMA slack around that
stream:

  P1: V = x Wv^T + bv -> vaug tiles (ones column for the softmax denom),
      with K^T/Q^T j=0 projections interleaved as their DMAs land.
  P2: per (q-block, head, t2-chunk):
        S^T[t2,q] = Kpad_h^T.T @ Q^T      (PSUM, "s" 2-slot rotation)
        P_raw     = exp(S^T / 8)          (ScalarE, the critical stream)
        P         = P_raw * (1-mask)^T    (DVE / GpSimd, alternating)
        O_aug^T  += Vaug_h.T @ P          (PSUM "o" banks, accum, lag 2)
      Remaining K/Q projections and P3(qb=0) y-tiles are injected into
      the "o"-parity PSUM banks mid-head so they never stall the "s"
      rotation that feeds the exp stream.
      Head epilogue (reciprocal of denom row + normalize) is emitted
      interleaved into the NEXT head's chunk loop so DVE never idles on
      its DMA-latency chain.
  P3: y^T = Wo^T.T @ O^T (+bo), per fo-tile, injected during the last
      heads of qb=1 (qb=0's during qb=1's early heads).

No max-subtraction in softmax: scores are O(6) so exp is safe in f32.
"""

import numpy as np
import ml_dtypes

B, T, FDIM, H = 8, 2048, 512, 8
DK = FDIM // H          # 64
NFT = FDIM // 128       # 4 fo-tiles
NCH = T // 128          # 16 t2-chunks
QB = 2                  # q blocks
QBS = T // QB           # 1024
N_CORES = 8
LAG = 2                 # attnV trails scores by LAG chunks

# mask-mult engine split: chunks with (c % GP_MOD) == GP_PHASE go to GpSimd
GP_MOD = 3
GP_PHASE = 1

BF16 = ml_dtypes.bfloat16

_cache = {}


def _build_nc():
    import concourse.bass as bass
    import concourse.mybir as mybir
    from concourse import bacc, tile

    f32 = mybir.dt.float32
    bf16 = mybir.dt.bfloat16
    Exp = mybir.ActivationFunctionType.Exp
    Alu = mybir.AluOpType

    nc = bacc.Bacc("TRN2", target_bir_lowering=False, debug=False,
                   num_devices=N_CORES)

    # DRAM I/O (per-core shard shapes)
    xqT = nc.dram_tensor("xqT", [FDIM, T], bf16, kind="ExternalInput")
    xkT = nc.dram_tensor("xkT", [FDIM, T], bf16, kind="ExternalInput")
    xvT = nc.dram_tensor("xvT", [FDIM, T], bf16, kind="ExternalInput")
    wqT = nc.dram_tensor("wqT", [FDIM, FDIM], bf16, kind="ExternalInput")
    wkT = nc.dram_tensor("wkT", [FDIM, FDIM], bf16, kind="ExternalInput")
    wvT = nc.dram_tensor("wvT", [FDIM, FDIM], bf16, kind="ExternalInput")
    woT = nc.dram_tensor("woT", [FDIM, FDIM], bf16, kind="ExternalInput")
    bq = nc.dram_tensor("bq", [FDIM], f32, kind="ExternalInput")
    bk = nc.dram_tensor("bk", [FDIM], f32, kind="ExternalInput")
    bv = nc.dram_tensor("bv", [FDIM], f32, kind="ExternalInput")
    bo = nc.dram_tensor("bo", [FDIM], f32, kind="ExternalInput")
    mbar = nc.dram_tensor("mbar", [NCH, 128, T], bf16, kind="ExternalInput")
    yT = nc.dram_tensor("yT", [FDIM, T], f32, kind="ExternalOutput")
    # DRAM bounce rows for partition-broadcasting softmax reciprocals
    rscratch = nc.dram_tensor("rscratch", [QB * H, QBS], f32)

    with tile.TileContext(nc) as tc:
        with (
            tc.tile_pool(name="consts", bufs=1) as consts,
            tc.tile_pool(name="qt", bufs=1) as qt_pool,
            tc.tile_pool(name="kpad", bufs=1) as kpad_pool,
            tc.tile_pool(name="vaug", bufs=1) as vaug_pool,
            tc.tile_pool(name="osb", bufs=1) as osb_pool,
            tc.tile_pool(name="ysb", bufs=1) as ysb_pool,
            tc.tile_pool(name="xt", bufs=4) as xt_pool,
            tc.tile_pool(name="mask", bufs=16) as mask_pool,
            tc.tile_pool(name="praw", bufs=3) as praw_pool,
            tc.tile_pool(name="pm", bufs=5) as pm_pool,
            tc.tile_pool(name="rb", bufs=1) as rb_pool,
            tc.tile_pool(name="psum", bufs=2, space="PSUM") as psum_pool,
        ):
            # ---- DMA wave 1: V path, then K path, then Q path ----
            wv_sb = [consts.tile([128, FDIM], bf16, tag=f"wv{fc}", name=f"wv{fc}") for fc in range(4)]
            wk_sb = [consts.tile([128, FDIM], bf16, tag=f"wk{fc}", name=f"wk{fc}") for fc in range(4)]
            wq_sb = [consts.tile([128, FDIM], bf16, tag=f"wq{fc}", name=f"wq{fc}") for fc in range(4)]
            wo_sb = [consts.tile([128, FDIM], bf16, tag=f"wo{j}", name=f"wo{j}") for j in range(NFT)]

            for fc in range(4):
                nc.sync.dma_start(out=wv_sb[fc][:], in_=wvT[fc * 128:(fc + 1) * 128, :])
            xts_v = []
            for fc in range(4):
                xt = xt_pool.tile([128, T], bf16, tag="xv", bufs=4, name="xv")
                nc.sync.dma_start(out=xt[:], in_=xvT[fc * 128:(fc + 1) * 128, :])
                xts_v.append(xt)
            for fc in range(4):
                nc.sync.dma_start(out=wk_sb[fc][:], in_=wkT[fc * 128:(fc + 1) * 128, :])
            xts_k = []
            for fc in range(4):
                xt = xt_pool.tile([128, T], bf16, tag="xk", bufs=4, name="xk")
                nc.sync.dma_start(out=xt[:], in_=xkT[fc * 128:(fc + 1) * 128, :])
                xts_k.append(xt)
            for fc in range(4):
                nc.sync.dma_start(out=wq_sb[fc][:], in_=wqT[fc * 128:(fc + 1) * 128, :])
            xts_q = []
            for fc in range(4):
                xt = xt_pool.tile([128, T], bf16, tag="xq", bufs=4, name="xq")
                nc.sync.dma_start(out=xt[:], in_=xqT[fc * 128:(fc + 1) * 128, :])
                xts_q.append(xt)
            for j in range(NFT):
                nc.sync.dma_start(out=wo_sb[j][:], in_=woT[j * 128:(j + 1) * 128, :])

            bq_sb = consts.tile([128, NFT], f32, tag="bq", name="bq")
            bk_sb = consts.tile([128, NFT], f32, tag="bk", name="bk")
            bo_sb = consts.tile([128, NFT], f32, tag="bo", name="bo")
            for b_dram, b_t in ((bq, bq_sb), (bk, bk_sb), (bo, bo_sb)):
                nc.sync.dma_start(out=b_t[:], in_=b_dram.ap().rearrange("(j p) -> p j", p=128))
            bv_bcast = consts.tile([128, FDIM], f32, tag="bv_bcast", name="bv_bcast")
            nc.sync.dma_start(
                out=bv_bcast[:],
                in_=bv.ap().rearrange("(a f) -> a f", a=1).to_broadcast([128, FDIM]))

            # ---- persistent activation tiles ----
            qT_sb = [qt_pool.tile([128, T], bf16, tag=f"qT{j}", name=f"qT{j}") for j in range(NFT)]
            kpad = [kpad_pool.tile([128, T], bf16, tag=f"kp{h}", name=f"kp{h}") for h in range(H)]
            for h in range(H):
                half = slice(64, 128) if h % 2 == 0 else slice(0, 64)
                nc.vector.memset(kpad[h][half, :], 0.0)
            vaug = [vaug_pool.tile([128, H * (DK + 1)], bf16, tag=f"va{tt}", name=f"va{tt}")
                    for tt in range(NCH)]
            for tt in range(NCH):
                va = vaug[tt][:].rearrange("p (h d) -> p h d", d=DK + 1)
                nc.vector.memset(va[:, :, DK:DK + 1], 1.0)

            o2_sb = {}
            for qb in range(QB):
                for j in range(NFT):
                    o2_sb[(qb, j)] = osb_pool.tile([128, QBS], bf16, tag=f"o2_{qb}_{j}",
                                                   name=f"o2_{qb}_{j}")

            # ---------------- unit emitters ----------------
            # PSUM: tag "s" = 2x[128,1024] (4 banks, the exp-feed rotation)
            #       tag o0/o1 = 1 each (2 banks each: o-accum / injected units)

            def v_unit(tt):
                # one V-projection t2-chunk -> vaug[tt]; psum parity-tagged
                ps = psum_pool.tile([128, 512], mybir.dt.float32,
                                    tag=f"o{tt % 2}", bufs=1, name="vp")
                for fc in range(4):
                    nc.tensor.matmul(
                        ps[:],
                        xts_v[fc][:, tt * 128:(tt + 1) * 128],
                        wv_sb[fc][:],
                        start=(fc == 0), stop=(fc == 3),
                    )
                va = vaug[tt][:].rearrange("p (h d) -> p h d", d=DK + 1)
                nc.vector.scalar_tensor_tensor(
                    out=va[:, :, 0:DK],
                    in0=ps[:].rearrange("p (h d) -> p h d", d=DK),
                    scalar=1.0,
                    in1=bv_bcast[:].rearrange("p (h d) -> p h d", d=DK),
                    op0=Alu.mult, op1=Alu.add,
                )

            def qk_unit(kind, j, s, ptag):
                # one [128,512] projection slice: Q^T/K^T features j*128.. for
                # t-cols s*512..; evicted with bias add.
                xts, w_sb, b_t = ((xts_q, wq_sb, bq_sb) if kind == "q"
                                  else (xts_k, wk_sb, bk_sb))
                ps = psum_pool.tile([128, 512], mybir.dt.float32,
                                    tag=ptag, bufs=(2 if ptag == "s" else 1),
                                    name="qkp")
                for fc in range(4):
                    nc.tensor.matmul(
                        ps[:],
                        w_sb[fc][:, j * 128:(j + 1) * 128],
                        xts[fc][:, s * 512:(s + 1) * 512],
                        start=(fc == 0), stop=(fc == 3),
                    )
                sl = slice(s * 512, (s + 1) * 512)
                if kind == "q":
                    nc.vector.tensor_scalar_add(qT_sb[j][:, sl], ps[:],
                                                b_t[:, j:j + 1])
                else:
                    nc.vector.tensor_scalar_add(
                        kpad[2 * j][0:64, sl], ps[0:64, :], b_t[0:64, j:j + 1])
                    nc.vector.tensor_scalar_add(
                        kpad[2 * j + 1][64:128, sl], ps[64:128, :],
                        b_t[64:128, j:j + 1])

            def p3_unit(qb, i, ptag):
                # one output-projection fo-tile: y^T rows i*128.. for q-block qb
                qsl = slice(qb * QBS, (qb + 1) * QBS)
                y_ps = psum_pool.tile([128, QBS], mybir.dt.float32,
                                      tag=ptag, bufs=1, name="y")
                for j in range(NFT):
                    for s in range(2):
                        nc.tensor.matmul(
                            y_ps[:, s * 512:(s + 1) * 512],
                            wo_sb[j][:, i * 128:(i + 1) * 128],
                            o2_sb[(qb, j)][:, s * 512:(s + 1) * 512],
                            start=(j == 0), stop=(j == NFT - 1),
                        )
                y_sb = ysb_pool.tile([128, QBS], mybir.dt.float32, tag="ysb", name="ysb")
                nc.vector.tensor_scalar_add(y_sb[:], y_ps[:], bo_sb[:, i:i + 1])
                nc.sync.dma_start(out=yT[i * 128:(i + 1) * 128, qsl], in_=y_sb[:])

            # ---------------- P1 ----------------
            # V projection units with K/Q j=0 units interleaved so the first
            # exp can issue as soon as kpad[0]/qT[0](qb0) are ready.
            p1_stream = []
            for tt in range(NCH):
                p1_stream.append(("v", tt))
            # K j=0 (full T) inserted after enough vproj to let xk land
            p1_stream[4:4] = [("k", 0, 0), ("k", 0, 1), ("k", 0, 2), ("k", 0, 3)]
            # Q j=0 first half (qb0) a bit later (xq lands after xk)
            p1_stream[11:11] = [("q", 0, 0), ("q", 0, 1)]
            for u in p1_stream:
                if u[0] == "v":
                    v_unit(u[1])
                else:
                    kind, j, s = u
                    qk_unit(kind, j, s, ptag="s")

            # ---------------- P2 ----------------
            # Injection schedule: per (qb, h) a list of units to emit inside
            # the chunk loop. Each unit ~1us PE (p3 units ~2.2us).
            inj = {qb: {h: [] for h in range(H)} for qb in range(QB)}
            # K(j) before head 2j; Q(j, half) before (qb=half, head 2j)
            inj[0][0] = [("k", 1, 0), ("k", 1, 1)]
            inj[0][1] = [("k", 1, 2), ("k", 1, 3), ("q", 1, 0), ("q", 1, 1)]
            inj[0][2] = [("k", 2, 0), ("k", 2, 1)]
            inj[0][3] = [("k", 2, 2), ("k", 2, 3), ("q", 2, 0), ("q", 2, 1)]
            inj[0][4] = [("k", 3, 0), ("k", 3, 1)]
            inj[0][5] = [("k", 3, 2), ("k", 3, 3), ("q", 3, 0), ("q", 3, 1)]
            inj[0][6] = [("q", 0, 2), ("q", 0, 3)]
            inj[1][0] = [("q", 1, 2), ("q", 1, 3)]
            inj[1][1] = [("q", 2, 2), ("q", 2, 3)]
            inj[1][2] = [("q", 3, 2), ("q", 3, 3)]
            inj[1][3] = [("p3", 0, 0)]
            inj[1][4] = [("p3", 0, 1)]
            inj[1][5] = [("p3", 0, 2)]
            inj[1][6] = [("p3", 0, 3)]

            mask_t = {}          # (qb, c) -> tile
            pending_epi = []     # stages of the previous head's epilogue

            def load_masks(qb, cs):
                for c in cs:
                    mt = mask_pool.tile([128, QBS], bf16, tag="mask", name="mask")
                    nc.sync.dma_start(out=mt[:], in_=mbar[c, :, qb * QBS:(qb + 1) * QBS])
                    mask_t[(qb, c)] = mt

            def epilogue_stages(qb, h, o_ps):
                # Returns a list of closures; each is one interleave step of
                # the head's softmax-normalization epilogue.
                j, lo = h // 2, (h % 2) * 64
                rrow = rscratch.ap()[qb * H + h: qb * H + h + 1, :]
                rb = rb_pool.tile([128, QBS], mybir.dt.float32, tag="rb", bufs=1, name="rb")
                rbs = rb_pool.tile([8, QBS // 8], mybir.dt.float32, tag="rbs", bufs=1, name="rbs")
                rbr = rb_pool.tile([8, QBS // 8], mybir.dt.float32, tag="rbr", bufs=1, name="rbr")

                def s1():
                    nc.vector.tensor_copy(rb[64:65, :], o_ps[DK:DK + 1, :])
                    nc.sync.dma_start(out=rbs[:], in_=rb[64:65, :])

                def s2():
                    nc.vector.reciprocal_approx_fast(rbr[:], rbs[:])
                    nc.sync.dma_start(out=rrow, in_=rbr[:])
                    nc.sync.dma_start(out=rb[0:64, :],
                                      in_=rrow.to_broadcast([64, QBS]))

                def s3():
                    if lo == 0:
                        nc.vector.tensor_mul(o2_sb[(qb, j)][0:64, :],
                                             o_ps[0:DK, :], rb[0:64, :])
                    else:
                        osm = rb_pool.tile([64, QBS], bf16, tag="osm", bufs=1, name="osm")
                        nc.vector.tensor_mul(osm[:], o_ps[0:DK, :], rb[0:64, :])
                        nc.sync.dma_start(out=o2_sb[(qb, j)][64:128, :], in_=osm[:])

                return [s1, s2, s3]

            load_masks(0, range(NCH))

            for qb in range(QB):
                qsl = slice(qb * QBS, (qb + 1) * QBS)
                for h in range(H):
                    j = h // 2
                    o_ps = psum_pool.tile([DK + 1, QBS], mybir.dt.float32,
                                          tag=f"o{h % 2}", bufs=1, name="o")
                    inj_units = list(inj[qb][h])
                    # spread injections across chunks 5,7,9,... (after the
                    # previous head's epilogue has released its o-psum slot)
                    inj_at = {5 + 2 * i: u for i, u in enumerate(inj_units)}
                    epi = list(pending_epi)
                    pending_epi = []
                    # epilogue stages of the previous head at chunks 0,1,3
                    epi_at = {pos: s for pos, s in zip((0, 1, 3), epi)}

                    pm_tiles = {}

                    def attn_v(c):
                        p_m = pm_tiles.pop(c)
                        for s in range(2):
                            nc.tensor.matmul(
                                o_ps[:, s * 512:(s + 1) * 512],
                                vaug[c][:, h * (DK + 1):(h + 1) * (DK + 1)],
                                p_m[:, s * 512:(s + 1) * 512],
                                start=(c == 0), stop=(c == NCH - 1),
                            )

                    for c in range(NCH):
                        s_ps = psum_pool.tile([128, QBS], mybir.dt.float32,
                                              tag="s", name="s")
                        for s in range(2):
                            nc.tensor.matmul(
                                s_ps[:, s * 512:(s + 1) * 512],
                                kpad[h][:, c * 128:(c + 1) * 128],
                                qT_sb[j][:, qb * QBS + s * 512: qb * QBS + (s + 1) * 512],
                                start=True, stop=True,
                            )
                        p_raw = praw_pool.tile([128, QBS], bf16, tag="praw", name="praw")
                        nc.scalar.activation(p_raw[:], s_ps[:], Exp,
                                             bias=0.0, scale=0.125)
                        p_m = pm_pool.tile([128, QBS], bf16, tag="pm", name="pm")
                        eng = nc.gpsimd if (c % GP_MOD) == GP_PHASE else nc.vector
                        eng.tensor_mul(p_m[:], p_raw[:], mask_t[(qb, c)][:])
                        pm_tiles[c] = p_m

                        if c in epi_at:
                            epi_at[c]()
                        if c - LAG >= 0:
                            attn_v(c - LAG)
                        if c in inj_at:
                            u = inj_at[c]
                            if u[0] == "p3":
                                p3_unit(u[1], u[2], ptag=f"o{(h + 1) % 2}")
                            else:
                                qk_unit(u[0], u[1], u[2], ptag=f"o{(h + 1) % 2}")
                        # prefetch next q-block's first mask near the end
                        if qb == 0 and h == H - 1 and c == 10:
                            load_masks(1, range(0, 1))

                    for c in range(NCH - LAG, NCH):
                        attn_v(c)
                    pending_epi = epilogue_stages(qb, h, o_ps)

                if qb == 0:
                    load_masks(1, range(1, NCH))

            # drain the last head's epilogue
            for st in pending_epi:
                st()
            # ---------------- P3 for qb=1 ----------------
            for i in range(NFT):
                p3_unit(1, i, ptag=f"o{i % 2}")

    nc.compile()
    return nc


def _get_nc():
    if "nc" not in _cache:
        _cache["nc"] = _build_nc()
    return _cache["nc"]


def _make_in_maps(inputs):
    query = np.asarray(inputs["query"], np.float32)
    key = np.asarray(inputs["key"], np.float32)
    value = np.asarray(inputs["value"], np.float32)
    mask = np.asarray(inputs["mask"], bool)
    shared = {
        "wqT": np.ascontiguousarray(np.asarray(inputs["Wq"], np.float32).T).astype(BF16),
        "wkT": np.ascontiguousarray(np.asarray(inputs["Wk"], np.float32).T).astype(BF16),
        "wvT": np.ascontiguousarray(np.asarray(inputs["Wv"], np.float32).T).astype(BF16),
        "woT": np.ascontiguousarray(np.asarray(inputs["Wo"], np.float32).T).astype(BF16),
        "bq": np.asarray(inputs["bq"], np.float32),
        "bk": np.asarray(inputs["bk"], np.float32),
        "bv": np.asarray(inputs["bv"], np.float32),
        "bo": np.asarray(inputs["bo"], np.float32),
    }
    in_maps = []
    for b in range(N_CORES):
        m = dict(shared)
        m["xqT"] = np.ascontiguousarray(query[b].T).astype(BF16)
        m["xkT"] = np.ascontiguousarray(key[b].T).astype(BF16)
        m["xvT"] = np.ascontiguousarray(value[b].T).astype(BF16)
        mb = (~mask[b]).T.astype(BF16)          # (1 - mask)^T, [t2, q]
        m["mbar"] = np.ascontiguousarray(mb.reshape(NCH, 128, T))
        in_maps.append(m)
    return in_maps


def run(inputs, trace=False, **kwargs):
    from concourse.bass_utils import run_bass_kernel_spmd
    nc = _get_nc()
    res = run_bass_kernel_spmd(nc, _make_in_maps(inputs),
                               core_ids=list(range(N_CORES)),
                               trace=trace, **kwargs)
    y = np.stack([np.asarray(res.results[b]["yT"], np.float32).T
                  for b in range(N_CORES)])
    return y, res


def kernel(**inputs) -> np.ndarray:
    y, _ = run(inputs, trace=False)
    return y
